# revision 34
# baseline (speedup 1.0000x reference)
"""BlockStackingSGN kernel for 8 Trainium2 NeuronCores.

Data-parallel over batch B=4096 -> 512 rows/core; weights replicated.
All 256-contraction matmuls run as fp8e4 DoubleRow (two 128-k-subtiles per
instruction at bf16 per-instruction cost => 2x PE throughput). Activations
are stored fp8 at a fixed power-of-2 scale SH with weights unscaled, so
every scale telescopes and each PSUM evacuation is a single fused
bias+relu+cast op on either the Scalar or Vector engine (GpSimd cannot
access PSUM); the pair-phase add/relu run on Vector with overflow adds on
GpSimd, assigned by a greedy cost balancer. The three 256->1 heads
accumulate into one PSUM bank via sliding-window fp8 weight strips and a
single batched Sigmoid finishes the kernel. Activation SBUF layouts are
m-tile-major so evacuation destinations stay contiguous 2D slices and
DoubleRow operands are plain rearrange views.
"""

import sys

import numpy as np

sys.path.insert(0, "/opt/trn_rl_repo")

import concourse.bacc as bacc
import concourse.mybir as mybir
import concourse.tile as tile
from concourse.bass_utils import run_bass_kernel_spmd

dt = mybir.dt
AF = mybir.ActivationFunctionType
ALU = mybir.AluOpType
PM = mybir.MatmulPerfMode

N = 8          # blocks
H = 256        # hidden
B = 4096       # batch
IN = 3 * N     # 24
NCORES = 8
BC = B // NCORES   # 512 batch rows per core
R = N * (N + 2)    # 80 output rows per batch element
SH = 16.0          # activation fp8 scale (weights unscaled)

F32 = dt.float32
BF16 = dt.bfloat16
FP8 = dt.float8e4
W = BC

_CACHE = {}

# ---- wb8 fp8 weight tile column layout ----
OW1_OFF = 0                      # 8 blocks x 2 m-tiles x 256
OW2_OFF = OW1_OFF + 4096
CW0_OFF = OW2_OFF + 4096         # 2 m x 256 each
CW1_OFF = CW0_OFF + 512
TW0_OFF = CW1_OFF + 512
TW1_OFF = TW0_OFF + 512
AL_OFF = TW1_OFF + 512           # aW0 left half
AR_OFF = AL_OFF + 512
AW1_OFF = AR_OFF + 512
W2E_OFF = AW1_OFF + 512          # 3 strips x 512 + 128 pad
WB_COLS = W2E_OFF + 3 * 512 + 128

# ---- bias tile [128, NB] f32 (values pre-scaled by SH; finb raw) ----
def _bcols():
    cols = {}
    c = 0
    for nm, cnt in (("ob0", 16), ("ob1", 16), ("ob2", 16), ("cb0", 2), ("cb1", 2),
                    ("tb0", 2), ("tb1", 2), ("ab0", 2), ("ab1", 2), ("finb", 1)):
        cols[nm] = c
        c += cnt
    return cols, c


BCOL, NB = _bcols()


def _build():
    nc = bacc.Bacc("TRN2", target_bir_lowering=False, debug=False, num_devices=NCORES)

    d_xw0 = nc.dram_tensor("xw0", [IN, BC + N * H], BF16, kind="ExternalInput")
    d_wb8 = nc.dram_tensor("wb8", [128, WB_COLS], FP8, kind="ExternalInput")
    d_bias = nc.dram_tensor("bias", [128, NB], F32, kind="ExternalInput")
    d_out = nc.dram_tensor("outT", [R, BC], F32, kind="ExternalOutput")

    # greedy elementwise engine balancer (costs in ns, calibrated on hw)
    eng_ns = {"dve": 0.0, "act": 0.0, "pool": 0.0}

    def pick(cands):
        e, c = min(cands, key=lambda ec: eng_ns[ec[0]] + ec[1])
        eng_ns[e] += c
        return e

    with tile.TileContext(nc) as tc:
        with (
            tc.tile_pool(name="wp", bufs=1) as wp,
            tc.tile_pool(name="h0p", bufs=8) as h0p,
            tc.tile_pool(name="h1p", bufs=8) as h1p,
            tc.tile_pool(name="phsp", bufs=8) as phsp,
            tc.tile_pool(name="ph8p", bufs=8) as ph8p,
            tc.tile_pool(name="y2p", bufs=4) as y2p,
            tc.tile_pool(name="hp2p", bufs=3) as hp2p,
            tc.tile_pool(name="hp2bp", bufs=3) as hp2bp,
            tc.tile_pool(name="ps", bufs=3, space="PSUM") as ps,
            tc.tile_pool(name="fpp", bufs=1, space="PSUM") as fpp,
            tc.tile_pool(name="fps", bufs=1, space="PSUM") as fps,
        ):
            xw0 = wp.tile([IN, BC + N * H], BF16, tag="xw0")
            nc.sync.dma_start(xw0[:, :BC], d_xw0[:, :BC])
            nc.gpsimd.dma_start(xw0[:, BC:], d_xw0[:, BC:])
            bias = wp.tile([128, NB], F32, tag="bias")
            nc.sync.dma_start(bias[:], d_bias[:])
            wb8 = wp.tile([128, WB_COLS], FP8, tag="wb8")
            # chunked in use-order, all on the sync DMA queue
            bounds = [0, 2048, 4096, 6144, 8192, AL_OFF, W2E_OFF, WB_COLS]
            for a, b in zip(bounds[:-1], bounds[1:]):
                nc.sync.dma_start(wb8[:, a:b], d_wb8[:, a:b])

            xT = xw0[:, :BC]

            # warm the ACT tables while input DMAs land: a sigmoid then a
            # relu dummy pulls the table load(s) off the critical path (and
            # lets the table pass settle on a set covering both, if any)
            scr = wp.tile([128, 1], F32, tag="scr")
            nc.scalar.activation(scr[:], bias[:, 0:1], AF.Sigmoid)
            nc.scalar.activation(scr[:], bias[:, 0:1], AF.Relu)

            # PE-warming fillers: the PE pstate reaches full clock only
            # after ~3us of continuous execution; idle gaps reset it and
            # leave every matmul at the 1.2GHz mid state. These dummy DR
            # matmuls (into a never-read scratch bank) fill the PE's idle
            # gaps so real matmuls run at 2.4GHz.
            pewarm = fps.tile([128, 512], F32, tag="pewarm")

            def pe_warm(anchor_rhs):
                nc.tensor.matmul(pewarm[:], lhsT_dr(OW1_OFF), anchor_rhs,
                                 start=True, stop=True,
                                 perf_mode=PM.DoubleRow)

            def bcol(nm, i=0):
                c = BCOL[nm] + i
                return bias[:, c : c + 1]

            def lhsT_dr(off):
                return wb8[:, off : off + 256].rearrange("p (a m) -> p a m", a=2)

            def rhs3(t, c0):
                """[128,(2,512)] DR rhs from an m-major fp8 tile region."""
                return t[:, c0 : c0 + 1024].rearrange("p (a w) -> p a w", a=2)

            def rhs3_g(t, half, c0):
                """DR rhs from a global m-major tile: subtile a at
                half*4096... no -- m-halves are `half` apart."""
                return t[:].rearrange("p (m r) -> p m r", m=2)[:, :, c0 : c0 + 512]

            # ---- elementwise emission helpers ----
            ACT_W, DVE_W = 1190.0, 1283.0
            ACT_S, DVE_S = 686.0, 700.0
            DVE_ADD2 = 1140.0
            DVE_RELU2, ACT_RELU2 = 1160.0, 2040.0

            def add2_op(dst, a, b):
                eng_ns["dve"] += DVE_ADD2
                nc.vector.tensor_tensor(dst, a, b, ALU.add)

            def relu2_op(dst, src):
                e = pick([("dve", DVE_RELU2), ("act", ACT_RELU2)])
                if e == "act":
                    nc.scalar.activation(dst, src, AF.Relu)
                else:
                    nc.vector.tensor_scalar(dst, src, 0.0, 1.0, ALU.max, ALU.mult)

            def evac(dst, src, bias_ap, relu, wide):
                ca, cd = (ACT_W, DVE_W) if wide else (ACT_S, DVE_S)
                e = pick([("act", ca), ("dve", cd)])
                if e == "act":
                    if relu:
                        nc.scalar.activation(dst, src, AF.Relu,
                                             bias=bias_ap if bias_ap is not None else 0.0)
                    elif bias_ap is not None:
                        nc.scalar.activation(dst, src, AF.Identity, bias=bias_ap)
                    else:
                        nc.scalar.activation(dst, src, AF.Copy)
                else:
                    if relu:
                        b = bias_ap if bias_ap is not None else 0.0
                        nc.vector.tensor_scalar(dst, src, b, 0.0, ALU.add, ALU.max)
                    elif bias_ap is not None:
                        nc.vector.tensor_scalar(dst, src, bias_ap, 1.0, ALU.add,
                                                ALU.mult)
                    else:
                        nc.vector.tensor_copy(dst, src)

            # ---- object encoders, breadth-first ----
            h0s, h1s = [], []
            for n in range(N):
                pA = ps.tile([128, 1024], F32, tag="ps")
                for mt in range(2):
                    lo = BC + n * H + mt * 128
                    nc.tensor.matmul(pA[:, mt * 512 : (mt + 1) * 512],
                                     xw0[:, lo : lo + 128], xT,
                                     start=True, stop=True)
                h0 = h0p.tile([128, 1024], FP8, tag="h0")
                for mt in range(2):
                    evac(h0[:, mt * 512 : (mt + 1) * 512],
                         pA[:, mt * 512 : (mt + 1) * 512],
                         bcol("ob0", n * 2 + mt), True, False)
                h0s.append(h0)

            for n in range(N):
                pA = ps.tile([128, 1024], F32, tag="ps")
                for mt in range(2):
                    nc.tensor.matmul(pA[:, mt * 512 : (mt + 1) * 512],
                                     lhsT_dr(OW1_OFF + (n * 2 + mt) * 256),
                                     rhs3(h0s[n], 0), start=True, stop=True,
                                     perf_mode=PM.DoubleRow)
                h1 = h1p.tile([128, 1024], FP8, tag="h1")
                for mt in range(2):
                    evac(h1[:, mt * 512 : (mt + 1) * 512],
                         pA[:, mt * 512 : (mt + 1) * 512],
                         bcol("ob1", n * 2 + mt), True, False)
                h1s.append(h1)

            # enc stored m-major: [128, (m, n, 512)]
            enc = wp.tile([128, 8192], FP8, tag="enc")
            for n in range(N):
                pA = ps.tile([128, 1024], F32, tag="ps")
                for mt in range(2):
                    nc.tensor.matmul(pA[:, mt * 512 : (mt + 1) * 512],
                                     lhsT_dr(OW2_OFF + (n * 2 + mt) * 256),
                                     rhs3(h1s[n], 0), start=True, stop=True,
                                     perf_mode=PM.DoubleRow)
                for mt in range(2):
                    evac(enc[:, mt * 4096 + n * 512 : mt * 4096 + (n + 1) * 512],
                         pA[:, mt * 512 : (mt + 1) * 512],
                         bcol("ob2", n * 2 + mt), False, False)

            def enc3(n):
                return rhs3_g(enc, 4096, n * 512)

            # ---- al/ar (SH-scaled bf16, m-major [128,(m, n, 512)]);
            # ab0 folded into al ----
            al = wp.tile([128, 8192], BF16, tag="al")
            ar = wp.tile([128, 8192], BF16, tag="ar")
            for n in range(0, N, 2):
                for dst, off, bnm in ((al, AL_OFF, "ab0"), (ar, AR_OFF, None)):
                    for mt in range(2):
                        T = ps.tile([128, 1024], F32, tag="ps")
                        nc.tensor.matmul(T[:, :512], lhsT_dr(off + mt * 256),
                                         enc3(n), start=True, stop=True,
                                         perf_mode=PM.DoubleRow)
                        nc.tensor.matmul(T[:, 512:], lhsT_dr(off + mt * 256),
                                         enc3(n + 1), start=True, stop=True,
                                         perf_mode=PM.DoubleRow)
                        evac(dst[:, mt * 4096 + n * 512 : mt * 4096 + (n + 2) * 512],
                             T[:], bcol(bnm, mt) if bnm else None, False, True)

            def alr3(t, i):
                return rhs3_g(t, 4096, i * 512)

            def al_bcast(i, nj):
                """al block-i slice broadcast over the j axis: [128,2,nj,512]."""
                return alr3(al, i).unsqueeze(2).broadcast_to((128, 2, nj, 512))

            def ars(j, nj):
                """ar blocks j..j+nj-1: [128, 2(m), nj, 512]."""
                return ar[:].rearrange("p (m n w) -> p m n w", m=2, n=8)[
                    :, :, j : j + nj, :]

            # ---- fin head accumulation bank ----
            fin = fpp.tile([128, BC], F32, tag="fin")
            fin_ct = [0]
            N_FIN = N * N + 2 * N

            def fin_mm(head, r, rhs):
                s = W2E_OFF + head * 512 + 128 - r
                lhsT = wb8[:, s : s + 512].rearrange("p (a m) -> p a m", a=2)[:, :, 0:128]
                first = fin_ct[0] == 0
                fin_ct[0] += 1
                nc.tensor.matmul(fin[:], lhsT, rhs, start=first,
                                 stop=fin_ct[0] == N_FIN, perf_mode=PM.DoubleRow)

            # ---- pred thunks: two same-type predicates (blocks n, n+1),
            # split into layer0/layer1 halves to spread psum-pool bursts ----
            def pred_thunk(w0off, w1off, b0nm, b1nm, head, n):
                st = {}

                def go0():
                    hp2 = hp2p.tile([128, 2048], FP8, tag="hp2")
                    for mt in range(2):
                        T = ps.tile([128, 1024], F32, tag="ps")
                        nc.tensor.matmul(T[:, :512], lhsT_dr(w0off + mt * 256),
                                         enc3(n), start=True, stop=True,
                                         perf_mode=PM.DoubleRow)
                        nc.tensor.matmul(T[:, 512:], lhsT_dr(w0off + mt * 256),
                                         enc3(n + 1), start=True, stop=True,
                                         perf_mode=PM.DoubleRow)
                        evac(hp2[:, mt * 1024 : (mt + 1) * 1024], T[:],
                             bcol(b0nm, mt), True, True)
                    st["hp2"] = hp2

                def go1():
                    hp2 = st["hp2"]
                    hp2b = hp2bp.tile([128, 2048], FP8, tag="hp2b")
                    for mt in range(2):
                        T = ps.tile([128, 1024], F32, tag="ps")
                        nc.tensor.matmul(T[:, :512], lhsT_dr(w1off + mt * 256),
                                         rhs3_g(hp2, 1024, 0), start=True, stop=True,
                                         perf_mode=PM.DoubleRow)
                        nc.tensor.matmul(T[:, 512:], lhsT_dr(w1off + mt * 256),
                                         rhs3_g(hp2, 1024, 512), start=True, stop=True,
                                         perf_mode=PM.DoubleRow)
                        evac(hp2b[:, mt * 1024 : (mt + 1) * 1024], T[:],
                             bcol(b1nm, mt), True, True)
                    for bi in range(2):
                        fin_mm(head, (n + bi) * 10 + 8 + head,
                               rhs3_g(hp2b, 1024, bi * 512))

                return go0, go1

            preds = []
            for n in range(0, N, 2):
                preds.extend(pred_thunk(CW0_OFF, CW1_OFF, "cb0", "cb1", 0, n))
                preds.extend(pred_thunk(TW0_OFF, TW1_OFF, "tb0", "tb1", 1, n))

            # ---- all 64 pairs, in groups of 2 (same i, adjacent j): one
            # merged 2048-free add, relu split per pair so W1 of pair k
            # starts as soon as its half is ready;
            # ph layout [128, (m, j, 512)] ----
            pairs = [(i, j) for i in range(N) for j in range(N)]
            for g in range(0, 64, 2):
                # front-loaded: all pred halves (and their fin_mms) are
                # emitted by g=46 so the fin tail is set by the last pair
                if preds and ((g < 32 and g % 4 == 0) or 32 <= g < 48):
                    preds.pop(0)()
                (i1, j1), (i2, j2) = pairs[g], pairs[g + 1]
                assert i1 == i2 and j2 == j1 + 1
                phA = phsp.tile([128, 2048], BF16, tag="phs")
                add2_op(phA[:].rearrange("p (m j w) -> p m j w", m=2, j=2),
                        al_bcast(i1, 2), ars(j1, 2))
                # p8: [128, (m, j, 512)]
                p8 = ph8p.tile([128, 2048], FP8, tag="ph8")
                relu2_op(p8[:], phA[:])

                # y2 m-major: [128, (m, pair, 512)]
                y2 = y2p.tile([128, 2048], FP8, tag="y2")
                for mt in range(2):
                    T = ps.tile([128, 1024], F32, tag="ps")
                    nc.tensor.matmul(T[:, :512], lhsT_dr(AW1_OFF + mt * 256),
                                     rhs3_g(p8, 1024, 0), start=True, stop=True,
                                     perf_mode=PM.DoubleRow)
                    nc.tensor.matmul(T[:, 512:], lhsT_dr(AW1_OFF + mt * 256),
                                     rhs3_g(p8, 1024, 512), start=True,
                                     stop=True, perf_mode=PM.DoubleRow)
                    evac(y2[:, mt * 1024 : (mt + 1) * 1024], T[:],
                         bcol("ab1", mt), True, True)
                fin_mm(2, i1 * 10 + j1, rhs3_g(y2, 1024, 0))
                fin_mm(2, i2 * 10 + j2, rhs3_g(y2, 1024, 512))
                pe_warm(rhs3_g(p8, 1024, 0))
                pe_warm(rhs3_g(p8, 1024, 512))
            for t in preds:
                t()

            assert fin_ct[0] == N_FIN

            # ---- batched sigmoid over all 80 head rows + store ----
            outT = wp.tile([128, BC], F32, tag="outT")
            nc.scalar.activation(outT[:], fin[:], AF.Sigmoid,
                                 bias=bcol("finb"), scale=1.0 / SH)
            nc.sync.dma_start(d_out[:], outT[:R, :])

    nc.compile()
    return nc


def _prep_inputs(inputs):
    import ml_dtypes

    fp8 = ml_dtypes.float8_e4m3
    bf = ml_dtypes.bfloat16
    f32a = lambda a: np.asarray(a, dtype=np.float32)

    wb8v = np.zeros((128, WB_COLS), fp8)

    def put_dr(off, Wt, mt):
        """[256,256] weight -> DR block [128, 256] at off (m-tile mt)."""
        blk = np.empty((128, 256), np.float32)
        blk[:, :128] = Wt[0:128, mt * 128 : (mt + 1) * 128]
        blk[:, 128:] = Wt[128:256, mt * 128 : (mt + 1) * 128]
        wb8v[:, off : off + 256] = blk.astype(fp8)

    oW1 = f32a(inputs["o_W1"])
    oW2 = f32a(inputs["o_W2"])
    for n in range(N):
        for mt in range(2):
            put_dr(OW1_OFF + (n * 2 + mt) * 256, oW1[n], mt)
            put_dr(OW2_OFF + (n * 2 + mt) * 256, oW2[n], mt)
    for off, src in ((CW0_OFF, "c_W0"), (CW1_OFF, "c_W1"),
                     (TW0_OFF, "t_W0"), (TW1_OFF, "t_W1")):
        a = f32a(inputs[src])
        for mt in range(2):
            put_dr(off + mt * 256, a, mt)
    aW0 = f32a(inputs["a_W0"])
    for mt in range(2):
        put_dr(AL_OFF + mt * 256, aW0[:H], mt)
        put_dr(AR_OFF + mt * 256, aW0[H:], mt)
    aW1 = f32a(inputs["a_W1"])
    for mt in range(2):
        put_dr(AW1_OFF + mt * 256, aW1, mt)
    for t_, src in enumerate(("c_W2", "t_W2", "a_W2")):
        w2 = f32a(inputs[src])[:, 0]
        s = W2E_OFF + t_ * 512
        wb8v[:, s + 128] = w2[:128].astype(fp8)
        wb8v[:, s + 384] = w2[128:].astype(fp8)

    biasv = np.zeros((128, NB), np.float32)

    def putb(nm, i, vec):
        biasv[:, BCOL[nm] + i] = vec

    for n in range(N):
        for nm, src in (("ob0", "o_b0"), ("ob1", "o_b1"), ("ob2", "o_b2")):
            a = f32a(inputs[src])[n]
            for mt in range(2):
                putb(nm, n * 2 + mt, SH * a[mt * 128 : (mt + 1) * 128])
    for nm, src in (("cb0", "c_b0"), ("cb1", "c_b1"), ("tb0", "t_b0"),
                    ("tb1", "t_b1"), ("ab0", "a_b0"), ("ab1", "a_b1")):
        a = f32a(inputs[src])
        for mt in range(2):
            putb(nm, mt, SH * a[mt * 128 : (mt + 1) * 128])
    finb = np.zeros(128, np.float32)
    for i in range(N):
        finb[i * 10 : i * 10 + 8] = f32a(inputs["a_b2"])[0]
        finb[i * 10 + 8] = f32a(inputs["c_b2"])[0]
        finb[i * 10 + 9] = f32a(inputs["t_b2"])[0]
    putb("finb", 0, finb)

    ow0v = np.zeros((IN, N * H), bf)
    oW0 = f32a(inputs["o_W0"])
    for n in range(N):
        ow0v[:, n * H : (n + 1) * H] = oW0[n].astype(bf)

    xT = np.ascontiguousarray(f32a(inputs["x"]).T) * SH  # SH-prescaled
    common = {"wb8": wb8v, "bias": biasv}
    in_maps = []
    for c in range(NCORES):
        m = dict(common)
        xw0 = np.empty((IN, BC + N * H), bf)
        xw0[:, :BC] = xT[:, c * BC : (c + 1) * BC].astype(bf)
        xw0[:, BC:] = ow0v
        m["xw0"] = xw0
        in_maps.append(m)
    return in_maps


def run(inputs, trace=False, **kw):
    if "nc" not in _CACHE:
        _CACHE["nc"] = _build()
    nc = _CACHE["nc"]
    in_maps = _prep_inputs(inputs)
    res = run_bass_kernel_spmd(nc, in_maps, list(range(NCORES)), trace=trace, **kw)
    out = np.concatenate([res.results[c]["outT"].T for c in range(NCORES)], axis=0)
    return out.astype(np.float32), res


def kernel(**inputs) -> np.ndarray:
    out, _ = run(inputs, trace=False)
    return out

